# revision 28
# baseline (speedup 1.0000x reference)
"""GATv2Conv GNN message-passing kernel for 8 Trainium2 NeuronCores.

Single-launch design optimized for the slow host<->device link:
  * Host ships only compact raw data (~1.6MB/core) packed into 4 tensors:
    an int8 pack (node-feature shard and edge-attr stream, quantized with
    per-feature scales that fold exactly into the device-side weight pack;
    int8 -> bf16 conversion of +-127 is lossless so the only error is the
    rounding itself), int16 per-edge gather-index streams, a bf16 pack
    (weights, small consts), and a tiny f32 pack. Host does indexing/layout
    and O(weights) scale folds only; all O(N+E) compute runs on device.
  * Device phase A: each core projects its node shard through [Wl|Wr|Wres]
    (one matmul per 128-node window), writes xl rows to a DRAM bounce and xr
    rows to a local DRAM table, and accumulates per-graph xres sums. An
    AllGather publishes the full xl table (messages may source any node);
    xr/xres stay core-local because edges are sharded by destination graph.
  * Device edge loop (per dst window, pieces of <=15 128-edge chunks):
    dma_gather xl[src] from the allgathered table (split in two <32k-row
    halves to fit int16 gather indices; edges laid out lo-half-first per
    window), xr[dst] from the local table, and the scatter one-hot rows from
    a device-built identity table; s = xl + xr + ea*We via DVE; leaky-relu
    (ACT Prelu); logits = reduce(t*att); exp; msg = xl*exp; one-hot
    scatter-matmul into per-window PSUM accumulating the weighted message
    sum and the softmax denominator.
  * Per window: normalize, accumulate per-graph h and h^2 sums via one-hot
    matmul (one-hots built on device from a per-node graph-slot stream).
  * Tail (same launch): AllReduce the [128,1] BN partial sums, finish BN
    affine, add pooled residual, run the 2-layer MLP head per core for its
    own <=16 graphs. Output is [2,16] f32 per core; host reassembles [G,2].
"""

import os
import numpy as np
import ml_dtypes

os.environ.setdefault("NEURON_RT_RESET_CORES", "1")
os.environ.setdefault("CONCOURSE_SCRUB_NEFF_DEBUG_INFO", "1")
bf16 = ml_dtypes.bfloat16

P = 128
HEADS = 4
OUT_C = 16
D = 64
GSLOT = 16
NC = 8
NEG_SLOPE = 0.2
BN_EPS = 1e-5
PIECE = 8  # max 128-edge chunks per dma_gather (1024-entry index ring)

_prog_cache = {}


# --------------------------------------------------------------------------
# host prep (indexing / layout only)
# --------------------------------------------------------------------------

def _pieces(n):
    out = []
    while n > 0:
        m = min(PIECE, n)
        out.append(m)
        n -= m
    return out


def _prep(inputs):
    x = np.asarray(inputs["x"], np.float32)
    ei = np.asarray(inputs["edge_index"], np.int32)
    ea = np.asarray(inputs["edge_attr"], np.float32)
    batch = np.asarray(inputs["batch"], np.int32)
    N, IN_C = x.shape
    CHX = IN_C + 1
    G = int(batch.max()) + 1

    # self loops (edge_attr fill 1.0), sort by destination
    src = np.concatenate([ei[0], np.arange(N, dtype=np.int32)])
    dst = np.concatenate([ei[1], np.arange(N, dtype=np.int32)])
    eav = np.concatenate([ea[:, 0], np.ones(N, np.float32)])
    order = np.argsort(dst, kind="stable")
    ss, ds, es = src[order], dst[order], eav[order]
    ET = ss.shape[0]

    # contiguous graph ranges per core, balanced by edge count
    nb = np.searchsorted(batch, np.arange(G + 1))
    ecnt_g = np.bincount(batch[ds], minlength=G)
    csum = np.cumsum(ecnt_g)
    gb = [0]
    for k in range(1, NC):
        b = int(np.searchsorted(csum, ET * k / NC))
        gb.append(min(max(b, gb[-1] + 1), G - (NC - k)))
    gb.append(G)
    gb = np.array(gb, np.int64)

    cores = []
    Wmax = 1
    for k in range(NC):
        g0, g1 = int(gb[k]), int(gb[k + 1])
        assert g1 - g0 <= GSLOT, f"core {k} has {g1 - g0} graphs > {GSLOT}"
        n0, n1 = int(nb[g0]), int(nb[g1])
        e0, e1 = np.searchsorted(ds, [n0, n1])
        W = max(1, -(-(n1 - n0) // P))
        Wmax = max(Wmax, W)
        cores.append(dict(g0=g0, g1=g1, n0=n0, n1=n1, e0=int(e0), e1=int(e1)))

    WP = Wmax * P
    SPLIT = (NC // 2) * WP

    # padded global node id: core k's nodes live at rows [k*WP, k*WP+nloc)
    pid = np.zeros(N, np.int64)
    for k in range(NC):
        c = cores[k]
        pid[c["n0"]:c["n1"]] = k * WP + np.arange(c["n1"] - c["n0"])

    # per-core edge buckets (window, lo/hi table half), find CPWlo/CPWhi
    CPWlo = 0
    CPWhi = 0
    ebuf = []
    for k in range(NC):
        c = cores[k]
        e0, e1, n0 = c["e0"], c["e1"], c["n0"]
        rel = (ds[e0:e1] - n0).astype(np.int64)
        w_e = rel >> 7
        p_s = pid[ss[e0:e1]]
        hi = p_s >= SPLIT
        ordk = np.lexsort((hi, w_e))
        sk = np.where(hi, p_s - SPLIT, p_s)[ordk]
        relk = rel[ordk]
        ak = es[e0:e1][ordk]
        hik = hi[ordk]
        wk = w_e[ordk]
        nlo = np.bincount(wk[~hik], minlength=Wmax)
        nhi = np.bincount(wk[hik], minlength=Wmax)
        if nlo.max(initial=0):
            CPWlo = max(CPWlo, int(-(-nlo.max() // P)))
        if nhi.max(initial=0):
            CPWhi = max(CPWhi, int(-(-nhi.max() // P)))
        ebuf.append((sk, relk, ak, wk, nlo, nhi))

    CPWlo = max(CPWlo, 1)
    CPWhi = max(CPWhi, 1)
    CPW = CPWlo + CPWhi
    T = Wmax * CPW
    L = T * P

    # shared weight packs
    Wl = np.asarray(inputs["Wl"], np.float32)
    Wr = np.asarray(inputs["Wr"], np.float32)
    Wres = np.asarray(inputs["Wres"], np.float32)
    wpk = np.zeros((CHX, 3 * D), np.float32)
    wpk[:IN_C, 0:D] = Wl
    wpk[:IN_C, D:2 * D] = Wr
    wpk[:IN_C, 2 * D:3 * D] = Wres
    wpk[IN_C, 0:D] = np.asarray(inputs["bl"], np.float32)
    wpk[IN_C, D:2 * D] = np.asarray(inputs["br"], np.float32)
    wpk[IN_C, 2 * D:3 * D] = np.asarray(inputs["bres"], np.float32)

    att = np.asarray(inputs["att"], np.float32)
    We = np.asarray(inputs["We"], np.float32)

    # int8 quantization scales for x and edge_attr; dequant folds exactly
    # into the weight pack (int8 -> bf16 conversion of +-127 is lossless,
    # so the only error is the rounding itself)
    s_x = np.maximum(np.abs(x).max(axis=0), 1e-30) / 127.0   # per-feature
    s_e = max(float(np.abs(eav).max()), 1e-30) / 127.0
    wpk[:IN_C] *= s_x[:, None]
    wpk[IN_C] /= 127.0

    # bf16 misc pack [P, ...]: wpk | attc | wec*s_e | iotac | iota16 |
    # iotap | gslot | w1w2 (rows 0:65)
    o_wpk = 0
    o_att = o_wpk + 3 * D
    o_wec = o_att + D
    o_ioc = o_wec + D
    o_i16 = o_ioc + P
    o_iop = o_i16 + GSLOT
    o_gsl = o_iop + 1
    o_w12 = o_gsl + Wmax
    BCOLS = o_w12 + D + 2
    bpk = np.zeros((P, BCOLS), np.float32)
    bpk[0:CHX, o_wpk:o_wpk + 3 * D] = wpk
    bpk[:, o_att:o_att + D] = np.tile(att.reshape(1, D), (P, 1))
    bpk[:, o_wec:o_wec + D] = np.tile(We.reshape(1, D) * s_e, (P, 1))
    bpk[:, o_ioc:o_ioc + P] = np.tile(np.arange(P, dtype=np.float32), (P, 1))
    bpk[:, o_i16:o_i16 + GSLOT] = np.tile(np.arange(GSLOT, dtype=np.float32),
                                          (P, 1))
    bpk[:, o_iop] = np.arange(P, dtype=np.float32)
    bpk[0:D, o_w12:o_w12 + D] = np.asarray(inputs["W1"], np.float32)
    bpk[D, o_w12:o_w12 + D] = np.asarray(inputs["b1"], np.float32)
    bpk[0:D, o_w12 + D:o_w12 + D + 2] = np.asarray(inputs["W2"], np.float32)
    bpk[D, o_w12 + D:o_w12 + D + 2] = np.asarray(inputs["b2"], np.float32)

    fpk = np.zeros((D, GSLOT + 4), np.float32)
    fpk[:, GSLOT] = np.asarray(inputs["gamma"], np.float32)
    fpk[:, GSLOT + 1] = np.asarray(inputs["beta"], np.float32)
    fpk[:, GSLOT + 2] = BN_EPS

    cnt_g = (nb[1:] - nb[:-1]).astype(np.float32)

    starts_lo = (np.arange(Wmax) * CPW) * P
    starts_hi = (np.arange(Wmax) * CPW + CPWlo) * P

    in_maps = []
    for k in range(NC):
        c = cores[k]
        n0, n1 = c["n0"], c["n1"]
        nloc = n1 - n0
        sk, relk, ak, wk, nlo, nhi = ebuf[k]

        sizes = np.stack([nlo, nhi], 1).ravel()
        bstart = np.concatenate([[0], np.cumsum(sizes)[:-1]])
        bases = np.stack([starts_lo, starts_hi], 1).ravel()
        j = np.arange(sk.shape[0])
        bid = np.repeat(np.arange(2 * Wmax), sizes)
        slot = bases[bid] + (j - bstart[bid])

        i16 = np.zeros((2, L), np.int16)
        i16[0, slot] = sk.astype(np.int16)            # src (table-half local)
        # dst local; pads point at the zeroed row block past the window so
        # the device-derived one-hot index (dst - 128w) hits the zero row
        i16[1, :] = ((np.arange(L) // (CPW * P)) * P + P).astype(np.int16)
        i16[1, slot] = relk.astype(np.int16)

        # int8 payload: x shard (transposed, ones row = 127) | ea stream
        x8 = np.zeros((P, WP + T), np.int8)
        x8[:IN_C, :nloc] = np.round(x[n0:n1].T / s_x[:, None]).astype(np.int8)
        x8[IN_C, :nloc] = 127
        eas = np.zeros(L, np.float32)
        eas[slot] = ak
        x8[:, WP:] = np.round(eas / s_e).reshape(T, P).T.astype(np.int8)

        bpkc = bpk.copy()
        gsl = np.full(WP, -1.0, np.float32)
        gsl[:nloc] = (batch[n0:n1] - c["g0"]).astype(np.float32)
        bpkc[:, o_gsl:o_gsl + Wmax] = gsl.reshape(Wmax, P).T

        fpkc = fpk.copy()
        ng = c["g1"] - c["g0"]
        icnt = np.ones(GSLOT, np.float32)
        icnt[:ng] = 1.0 / np.maximum(cnt_g[c["g0"]:c["g1"]], 1.0)
        fpkc[:, 0:GSLOT] = np.tile(icnt.reshape(1, GSLOT), (D, 1))

        m = dict(
            t_x8=x8,
            t_i16=i16.reshape(2, L // 16, 16).transpose(2, 0, 1)
                     .reshape(16, 2 * (L // 16)).copy(),
            t_bfp=bpkc.astype(bf16),
            t_fpk=fpkc,
        )
        in_maps.append(m)

    meta = dict(N=N, IN_C=IN_C, CHX=CHX, G=G, Wmax=Wmax, WP=WP,
                CPWlo=CPWlo, CPWhi=CPWhi, CPW=CPW, T=T, L=L, gb=gb,
                offs=dict(wpk=o_wpk, att=o_att, wec=o_wec, ioc=o_ioc,
                          i16=o_i16, iop=o_iop, gsl=o_gsl, w12=o_w12,
                          bcols=BCOLS))
    return meta, in_maps


# --------------------------------------------------------------------------
# bass program (single launch, collectives inside)
# --------------------------------------------------------------------------

def _build(meta, leaky_mode="prelu", debug=False):
    import concourse.bacc as bacc
    import concourse.mybir as mybir
    import concourse.tile as tile

    F32 = mybir.dt.float32
    BF = mybir.dt.bfloat16
    I16 = mybir.dt.int16
    AL = mybir.AluOpType
    AF = mybir.ActivationFunctionType
    AX = mybir.AxisListType

    N = meta["N"]
    CHX = meta["CHX"]
    Wmax, WP = meta["Wmax"], meta["WP"]
    CPWlo, CPWhi, CPW = meta["CPWlo"], meta["CPWhi"], meta["CPW"]
    T, L = meta["T"], meta["L"]
    SPLIT = (NC // 2) * WP
    O = meta["offs"]

    nc = bacc.Bacc(None, target_bir_lowering=False, num_devices=NC, debug=debug)

    I8 = mybir.dt.int8
    t_x8 = nc.dram_tensor("t_x8", [P, WP + T], I8, kind="ExternalInput")
    t_i16 = nc.dram_tensor("t_i16", [16, 2 * (L // 16)], I16,
                           kind="ExternalInput")
    t_bfp = nc.dram_tensor("t_bfp", [P, O["bcols"]], BF, kind="ExternalInput")
    t_fpk = nc.dram_tensor("t_fpk", [D, GSLOT + 4], F32, kind="ExternalInput")

    o_out = nc.dram_tensor("o_out", [2, GSLOT], F32, kind="ExternalOutput")

    xl_sh = nc.dram_tensor("xl_sh", [WP, D], F32)
    xl_full = nc.dram_tensor("xl_full", [NC * WP, D], F32)
    xr_tab = nc.dram_tensor("xr_tab", [WP + P, D], F32)
    oh_tab = nc.dram_tensor("oh_tab", [2 * P, P], BF)
    bn_in = nc.dram_tensor("bn_in", [P, 1], F32)
    bn_out = nc.dram_tensor("bn_out", [P, 1], F32)

    PL = _pieces(CPWlo)
    PH = _pieces(CPWhi)

    with tile.TileContext(nc) as tc:
        with tc.tile_pool(name="cst", bufs=1) as cst, \
             tc.tile_pool(name="pa", bufs=2, space="PSUM") as pa_pool, \
             tc.tile_pool(name="win", bufs=2, space="PSUM") as win_pool, \
             tc.tile_pool(name="acc", bufs=1, space="PSUM") as acc_pool, \
             tc.tile_pool(name="rsm", bufs=1, space="PSUM") as rsm_pool, \
             tc.tile_pool(name="gat", bufs=3) as gatp, \
             tc.tile_pool(name="wrk", bufs=3) as wrk:

            x8_t = cst.tile([P, WP + T], mybir.dt.int8, tag="x8")
            nc.sync.dma_start(x8_t[:], t_x8[:])
            xbf_t = cst.tile([P, WP + T], BF, tag="xbf")
            nc.vector.tensor_copy(xbf_t[:], x8_t[:])
            bfp_t = cst.tile([P, O["bcols"]], BF, tag="bfp")
            nc.sync.dma_start(bfp_t[:], t_bfp[:])
            fpk_t = cst.tile([D, GSLOT + 4], F32, tag="fpk")
            nc.sync.dma_start(fpk_t[:], t_fpk[:])
            # gather index streams: replicate [16, .] across the 8 gpsimd
            # stripes on device
            srct = cst.tile([P, L // 16], I16, tag="srct")
            dstt = cst.tile([P, L // 16], I16, tag="dstt")
            for r in range(8):
                nc.sync.dma_start(srct[16 * r:16 * (r + 1), :],
                                  t_i16[:, 0:L // 16])
                nc.sync.dma_start(dstt[16 * r:16 * (r + 1), :],
                                  t_i16[:, L // 16:2 * (L // 16)])
            # one-hot gather indices: dst_local - 128*window (pads land on
            # the zeroed row block at 128)
            reltt = cst.tile([P, L // 16], I16, tag="reltt")
            for w in range(Wmax):
                cs = w * CPW * 8
                nc.vector.tensor_scalar(reltt[:, cs:cs + CPW * 8],
                                        dstt[:, cs:cs + CPW * 8],
                                        float(-w * P), None, AL.add)

            ea_v = xbf_t[:, WP:WP + T]
            attc_t = cst.tile([P, PIECE, D], BF, tag="attc")
            nc.scalar.activation(
                attc_t[:], bfp_t[:, O["att"]:O["att"] + D].unsqueeze(1)
                .to_broadcast([P, PIECE, D]), AF.Copy)
            wec_t = cst.tile([P, PIECE, D], BF, tag="wecc")
            nc.scalar.activation(
                wec_t[:], bfp_t[:, O["wec"]:O["wec"] + D].unsqueeze(1)
                .to_broadcast([P, PIECE, D]), AF.Copy)
            attc_v = attc_t[:].rearrange("p c f -> p (c f)")
            wec_v = wec_t[:].rearrange("p c f -> p (c f)")
            iotac_v = bfp_t[:, O["ioc"]:O["ioc"] + P]
            iota16_v = bfp_t[:, O["i16"]:O["i16"] + GSLOT]
            w1_v = bfp_t[0:D + 1, O["w12"]:O["w12"] + D]
            w2_v = bfp_t[0:D + 1, O["w12"] + D:O["w12"] + D + 2]
            icnt_v = fpk_t[:, 0:GSLOT]
            misc_v = fpk_t[:, GSLOT:GSLOT + 4]

            iopf = cst.tile([P, 1], F32, tag="iopf")
            nc.vector.tensor_copy(iopf[:], bfp_t[:, O["iop"]:O["iop"] + 1])
            gslf_t = cst.tile([P, Wmax], F32, tag="gslf")
            nc.vector.tensor_copy(gslf_t[:], bfp_t[:, O["gsl"]:O["gsl"] + Wmax])
            gm_all = cst.tile([P, Wmax, GSLOT], BF, tag="gmall")

            # one-hot gather table: identity rows then a zero row block
            idt = wrk.tile([P, P], BF, tag="idt")
            nc.vector.tensor_scalar(idt[:], iotac_v, iopf[:], None, AL.is_equal)
            nc.gpsimd.dma_start(oh_tab[0:P, :], idt[:])
            zt = wrk.tile([P, P], BF, tag="zt")
            nc.vector.memset(zt[:], 0.0)
            nc.gpsimd.dma_start(oh_tab[P:2 * P, :], zt[:])
            ztf = wrk.tile([P, D], F32, tag="ztf")
            nc.vector.memset(ztf[:], 0.0)
            nc.gpsimd.dma_start(xr_tab[WP:WP + P, :], ztf[:])

            ps_rsum = rsm_pool.tile([D, GSLOT], F32, tag="rsum")
            ps_stats = acc_pool.tile([P, GSLOT], F32, tag="stats")

            # ---------------- phase A: projection tables -----------------
            for w in range(Wmax):
                ps_a = pa_pool.tile([P, 3 * D], F32, tag="pa")
                nc.tensor.matmul(ps_a[:], xbf_t[0:CHX, w * P:(w + 1) * P],
                                 bfp_t[0:CHX, O["wpk"]:O["wpk"] + 3 * D],
                                 start=True, stop=True,
                                 skip_group_check=True)
                sxl = wrk.tile([P, D], F32, tag="sxl")
                nc.scalar.activation(sxl[:], ps_a[:, 0:D], AF.Copy)
                nc.gpsimd.dma_start(xl_sh[w * P:(w + 1) * P, :], sxl[:])
                sxr = wrk.tile([P, D], F32, tag="sxr")
                nc.scalar.activation(sxr[:], ps_a[:, D:2 * D], AF.Copy)
                nc.gpsimd.dma_start(xr_tab[w * P:(w + 1) * P, :], sxr[:])
                sxe = wrk.tile([P, D], BF, tag="sxe")
                nc.scalar.activation(sxe[:], ps_a[:, 2 * D:3 * D], AF.Copy)
                nc.vector.tensor_scalar(gm_all[:, w, :], iota16_v,
                                        gslf_t[:, w:w + 1], None, AL.is_equal)
                nc.tensor.matmul(ps_rsum[:], sxe[:], gm_all[:, w, :],
                                 start=(w == 0), stop=(w == Wmax - 1),
                                 skip_group_check=True)

            nc.gpsimd.collective_compute(
                "AllGather", AL.bypass,
                replica_groups=[list(range(NC))],
                ins=[xl_sh[:]], outs=[xl_full[:]],
            )

            # ---------------- edge loop --------------------------------
            for w in range(Wmax):
                win_ps = win_pool.tile([P, D + HEADS], F32, tag="win")
                cw = 0  # chunk index within window
                for run_off, run_pieces, lo in ((0, PL, True),
                                                (CPWlo, PH, False)):
                    po = 0
                    for m in run_pieces:
                        c0 = w * CPW + run_off + po   # global chunk
                        so = c0 * P                   # global slot
                        gx = gatp.tile([P, m, D], F32, tag=f"gx{m}")
                        src_tab = xl_full[0:SPLIT, :] if lo \
                            else xl_full[SPLIT:2 * SPLIT, :]
                        nc.gpsimd.dma_gather(
                            out_ap=gx[:], in_ap=src_tab,
                            idxs_ap=srct[:, so // 16:(so + m * P) // 16],
                            num_idxs=m * P, num_idxs_reg=m * P, elem_size=D)
                        gr = gatp.tile([P, m, D], F32, tag=f"gr{m}")
                        nc.gpsimd.dma_gather(
                            out_ap=gr[:], in_ap=xr_tab[:],
                            idxs_ap=dstt[:, so // 16:(so + m * P) // 16],
                            num_idxs=m * P, num_idxs_reg=m * P, elem_size=D)
                        oh = gatp.tile([P, m, P], BF, tag=f"oh{m}")
                        nc.gpsimd.dma_gather(
                            out_ap=oh[:], in_ap=oh_tab[:],
                            idxs_ap=reltt[:, so // 16:(so + m * P) // 16],
                            num_idxs=m * P, num_idxs_reg=m * P, elem_size=P)

                        em = wrk.tile([P, m, D], F32, tag=f"em{m}")
                        nc.vector.tensor_tensor(
                            out=em[:],
                            in0=ea_v[:, c0:c0 + m].unsqueeze(2)
                                .to_broadcast([P, m, D]),
                            in1=wec_v[:, 0:m * D].rearrange(
                                "p (c f) -> p c f", c=m),
                            op=AL.mult)
                        sa = wrk.tile([P, m, D], F32, tag=f"sa{m}")
                        nc.vector.tensor_tensor(out=sa[:], in0=gx[:],
                                                in1=gr[:], op=AL.add)
                        nc.vector.tensor_tensor(out=sa[:], in0=sa[:],
                                                in1=em[:], op=AL.add)
                        sb_t = wrk.tile([P, m, D], BF, tag=f"t{m}")
                        if leaky_mode == "prelu":
                            nc.scalar.activation(sb_t[:], sa[:], AF.Prelu,
                                                 alpha=NEG_SLOPE)
                        else:
                            sb_r = wrk.tile([P, m, D], F32, tag=f"r{m}")
                            nc.scalar.activation(sb_r[:], sa[:], AF.Relu,
                                                 scale=-(1.0 - NEG_SLOPE))
                            nc.vector.tensor_tensor(out=sb_t[:], in0=sa[:],
                                                    in1=sb_r[:], op=AL.add)
                        sb_u = wrk.tile([P, m, D], BF, tag=f"u{m}")
                        nc.vector.tensor_tensor(
                            out=sb_u[:], in0=sb_t[:],
                            in1=attc_v[:, 0:m * D].rearrange(
                                "p (c f) -> p c f", c=m),
                            op=AL.mult)
                        sb_lg = wrk.tile([P, m, HEADS], F32, tag=f"lg{m}")
                        nc.vector.tensor_reduce(
                            out=sb_lg[:],
                            in_=sb_u[:].rearrange("p c (h k) -> p c h k",
                                                  k=OUT_C),
                            axis=AX.X, op=AL.add)
                        exf = wrk.tile([P, m, HEADS], F32, tag=f"ex{m}")
                        nc.scalar.activation(exf[:], sb_lg[:], AF.Exp)
                        exb = wrk.tile([P, m, D], F32, tag=f"exb{m}")
                        nc.scalar.activation(
                            exb[:].rearrange("p c (h k) -> p c h k", k=OUT_C),
                            exf[:].unsqueeze(3).to_broadcast(
                                [P, m, HEADS, OUT_C]),
                            AF.Copy)
                        msg = wrk.tile([P, m, D + HEADS], BF, tag=f"msg{m}")
                        nc.vector.tensor_tensor(out=msg[:, :, 0:D], in0=gx[:],
                                                in1=exb[:], op=AL.mult)
                        nc.scalar.activation(msg[:, :, D:D + HEADS], exf[:],
                                             AF.Copy)
                        for j in range(m):
                            nc.tensor.matmul(win_ps[:], oh[:, j, :],
                                             msg[:, j, :],
                                             start=(cw + j == 0),
                                             stop=(cw + j == CPW - 1),
                                             skip_group_check=True)
                        po += m
                        cw += m

                # window flush: softmax-normalize, accumulate BN/pool stats
                den = wrk.tile([P, HEADS], F32, tag="den")
                nc.vector.tensor_scalar(den[:], win_ps[:, D:D + HEADS],
                                        1e-20, None, AL.add)
                rd = wrk.tile([P, HEADS], F32, tag="rd")
                nc.vector.reciprocal(rd[:], den[:])
                hh2 = wrk.tile([P, 2 * D], BF, tag="hh2")
                nc.vector.tensor_tensor(
                    out=hh2[:, 0:D].rearrange("p (h k) -> p h k", k=OUT_C),
                    in0=win_ps[:, 0:D].rearrange("p (h k) -> p h k", k=OUT_C),
                    in1=rd[:].unsqueeze(2).to_broadcast([P, HEADS, OUT_C]),
                    op=AL.mult)
                nc.scalar.activation(hh2[:, D:2 * D], hh2[:, 0:D], AF.Square)
                nc.tensor.matmul(ps_stats[:], hh2[:], gm_all[:, w, :],
                                 start=(w == 0), stop=(w == Wmax - 1),
                                 skip_group_check=True)

            # ---------------- BN allreduce + tail ----------------------
            sl = wrk.tile([P, 1], F32, tag="sl")
            nc.vector.tensor_reduce(out=sl[:], in_=ps_stats[:],
                                    axis=AX.X, op=AL.add)
            nc.gpsimd.dma_start(bn_in[:], sl[:])
            nc.gpsimd.collective_compute(
                "AllReduce", AL.add,
                replica_groups=[list(range(NC))],
                ins=[bn_in[:]], outs=[bn_out[:]],
            )
            sh = wrk.tile([D, 2], F32, tag="sh")
            nc.gpsimd.dma_start(sh[:, 0:1], bn_out[0:D, :])
            nc.gpsimd.dma_start(sh[:, 1:2], bn_out[D:2 * D, :])

            mu = wrk.tile([D, 1], F32, tag="mu")
            nc.scalar.activation(mu[:], sh[:, 0:1], AF.Copy, scale=1.0 / N)
            e2 = wrk.tile([D, 1], F32, tag="e2")
            nc.scalar.activation(e2[:], sh[:, 1:2], AF.Copy, scale=1.0 / N)
            mu2 = wrk.tile([D, 1], F32, tag="mu2")
            nc.scalar.activation(mu2[:], mu[:], AF.Square)
            var = wrk.tile([D, 1], F32, tag="var")
            nc.vector.tensor_tensor(out=var[:], in0=e2[:], in1=mu2[:],
                                    op=AL.subtract)
            sd = wrk.tile([D, 1], F32, tag="sd")
            nc.scalar.activation(sd[:], var[:], AF.Sqrt, bias=misc_v[:, 2:3])
            rsd = wrk.tile([D, 1], F32, tag="rsd")
            nc.vector.reciprocal(rsd[:], sd[:])
            cA = wrk.tile([D, 1], F32, tag="cA")
            nc.vector.tensor_tensor(out=cA[:], in0=misc_v[:, 0:1], in1=rsd[:],
                                    op=AL.mult)
            tmp = wrk.tile([D, 1], F32, tag="tmp")
            nc.vector.tensor_tensor(out=tmp[:], in0=cA[:], in1=mu[:],
                                    op=AL.mult)
            cB = wrk.tile([D, 1], F32, tag="cB")
            nc.vector.tensor_tensor(out=cB[:], in0=misc_v[:, 1:2], in1=tmp[:],
                                    op=AL.subtract)

            ph = wrk.tile([D, GSLOT], F32, tag="ph")
            nc.vector.tensor_tensor(out=ph[:], in0=ps_stats[0:D, :],
                                    in1=icnt_v, op=AL.mult)
            pooled = wrk.tile([D, GSLOT], F32, tag="pooled")
            nc.vector.tensor_scalar(pooled[:], ph[:], cA[:], cB[:],
                                    AL.mult, AL.add)
            pr = wrk.tile([D, GSLOT], F32, tag="pr")
            nc.vector.tensor_tensor(out=pr[:], in0=ps_rsum[:], in1=icnt_v,
                                    op=AL.mult)
            zr = wrk.tile([D + 1, GSLOT], BF, tag="zr")
            nc.vector.memset(zr[D:D + 1, :], 1.0)
            nc.vector.tensor_tensor(out=zr[0:D, :], in0=pooled[:], in1=pr[:],
                                    op=AL.add)
            ps_z = win_pool.tile([D, GSLOT], F32, tag="win")
            nc.tensor.matmul(ps_z[:], w1_v, zr[:], start=True, stop=True,
                             skip_group_check=True)
            z2 = wrk.tile([D + 1, GSLOT], BF, tag="z2")
            nc.vector.memset(z2[D:D + 1, :], 1.0)
            nc.scalar.activation(z2[0:D, :], ps_z[:], AF.Relu)
            ps_o = win_pool.tile([2, GSLOT], F32, tag="win")
            nc.tensor.matmul(ps_o[:], w2_v, z2[:], start=True, stop=True,
                             skip_group_check=True)
            so = wrk.tile([2, GSLOT], F32, tag="so")
            nc.scalar.activation(so[:], ps_o[:], AF.Copy)
            nc.sync.dma_start(o_out[:], so[:])

    nc.compile()
    return nc


# --------------------------------------------------------------------------
# entry point
# --------------------------------------------------------------------------

def _run_sim(nc, in_maps):
    from concourse.bass_interp import MultiCoreSim
    ms = MultiCoreSim(nc, num_cores=NC, num_workers=NC,
                      require_finite=False, require_nnan=False)
    for k in range(NC):
        for name, arr in in_maps[k].items():
            ms.cores[k].tensor(name)[:] = arr
    ms.simulate()
    return [{"o_out": np.array(ms.cores[k].tensor("o_out"))} for k in range(NC)]


def _install_cc_cache():
    """Content-keyed memo around the neuronx compile hook.

    run_bass_kernel_spmd rebuilds its jax.jit closure per call, so jax's
    in-memory executable cache (weakref-keyed on the computation object)
    can never hit and the identical HLO is re-lowered through walrus every
    launch. Wrapping the hook with a hash(code)-keyed cache restores the
    caching jax would do if the backend supported serialized executables.
    """
    global _cc_cache_installed
    if _cc_cache_installed:
        return
    try:
        import hashlib
        from concourse import bass2jax

        inner = bass2jax.neuronx_cc_hook
        cache = {}

        def _canon(code):
            # jax bumps HloModuleProto.id per trace; it's the only byte that
            # differs between identical relaunches — zero it for the key
            try:
                import libneuronxla.proto.hlo_pb2 as hp
                p = hp.HloModuleProto.FromString(bytes(code))
                p.id = 0
                return p.SerializeToString()
            except Exception:
                return bytes(code)

        def cached_cc(code, code_format, platform_version, file_prefix):
            key = (hashlib.sha256(_canon(code)).digest(), bytes(code_format),
                   str(platform_version))
            if key not in cache:
                cache[key] = inner(code, code_format, platform_version,
                                   file_prefix)
            return cache[key]

        bass2jax.neuronx_cc_hook = cached_cc
        _cc_cache_installed = True
    except Exception:
        pass


_cc_cache_installed = False


def kernel(**inputs):
    meta, in_maps = _prep(inputs)
    key = ("v5", meta["IN_C"], meta["Wmax"], meta["CPWlo"], meta["CPWhi"],
           meta["N"], _LEAKY_MODE)
    if key not in _prog_cache:
        _prog_cache[key] = _build(meta, leaky_mode=_LEAKY_MODE,
                                  debug=(_RUN_MODE == "sim"))
    nc = _prog_cache[key]

    global LAST_EXEC_NS
    if _RUN_MODE == "sim":
        res = _run_sim(nc, in_maps)
        LAST_EXEC_NS = [None]
    else:
        from concourse.bass_utils import run_bass_kernel_spmd
        import time as _time
        _install_cc_cache()
        _t0 = _time.time()
        r = run_bass_kernel_spmd(nc, in_maps, list(range(NC)))
        _t1 = _time.time()
        res = r.results
        LAST_EXEC_NS = [getattr(r, "exec_time_ns", None)
                        or int((_t1 - _t0) * 1e9)]

    G = meta["G"]
    gb = meta["gb"]
    out = np.zeros((G, 2), np.float32)
    for g in range(G):
        k = int(np.searchsorted(gb, g, side="right")) - 1
        slot = g - int(gb[k])
        out[g] = res[k]["o_out"][:, slot]
    return out


_LEAKY_MODE = "prelu"
_RUN_MODE = "hw"
LAST_EXEC_NS = None
